# revision 56
# baseline (speedup 1.0000x reference)
"""Trainium2 Bass kernel for nn_Dyanmic_Q_MLP (fake-quant MLP).

Computation (reference):
    w1q = fake_quant(w1, 8); w2q = fake_quant(w2, 8)       # per-tensor symmetric
    h   = relu(x @ w1q.T + b1)                             # [B,S,3072]
    out = h @ w2q.T + b2                                   # [B,S,768]

Strategy (v2 — single-pass matmuls, ~2x the v1 hi/lo-split kernel):
  * Data-parallel over the flattened (B*S)=12544 rows across 8 NeuronCores
    (1568 rows/core).  Weights replicated.  No collectives.  Host side only
    reshapes/transposes/shards (layout, no math).
  * On-device fake-quant: per-partition abs-max (DVE reduce) while the
    weights stream in, replicated across partitions via exact PE f32
    transposes; integer-valued weights q = round(w/s) via the +-1.5*2^23
    RNE trick.
  * fc1 runs on the f32r PE path: w1 is DMAd ONCE into SBUF (f32,
    resident), quantized IN PLACE (q in [-127,127] is exact in f32r's
    mantissa), and both operands are bitcast to float32r.  With a moving
    free dim >= 256 f32r streams 1 row/cycle (same as bf16), so fc1 costs
    one pass and x needs no bf16 split ops at all (x error ~2^-12).
  * fc2 runs on the bf16 path: h is produced directly as bf16 by the fc1
    epilogue (one ACT op: relu(psum + b1/s1) -> bf16), w2q is quantized
    into bf16 tiles (ints exact).  h's bf16 rounding (~1.1e-3 rel) is the
    dominant error term; total rel err vs the fp32 reference ~1.2e-3.
  * Scales fold into the epilogues: relu(s1*z+b1) = s1*relu(z+b1/s1);
    out = (s1*s2)*psum + b2 fused into one ACT op.
  * Schedule: w1 scan gates everything, so w1's max-reduce pipelines
    behind its DMA (26us at 360GB/s), the in-place quantize is split
    DVE/Pool j-major and interleaved with fc1 block-0 groups, and fc1
    runs ahead by 3 blocks before fc2(b0) so the PE never waits for
    w2's (off-critical-path) 2-pass stream.  PE busy ~189us of ~222us.
"""

import sys

for _p in ("/opt/trn_rl_repo", "/root/.axon_site/_ro/trn_rl_repo"):
    if _p not in sys.path:
        sys.path.insert(0, _p)

from contextlib import ExitStack

import numpy as np

import concourse.bass as bass
import concourse.mybir as mybir
import concourse.tile as tile
from concourse import bass_isa, bass_utils
from concourse.tile_rust import add_dep_helper

N_CORES = 8
B, S, D, H = 64, 196, 768, 3072
M_TOTAL = B * S            # 12544
M_SHARD = M_TOTAL // N_CORES   # 1568
M_BLOCKS = [392, 392, 392, 392]
KD = D // 128              # 6
KH = H // 128              # 24
C_RNE = 12582912.0         # 1.5 * 2**23: (v + C) - C == round-to-nearest-even(v)
W1_SCAN_CHUNK = 768        # w1 DMA/scan slice width (24 slices, pipelines w/ DMA)
QJ = 384                   # w1 in-place quantize chunk width (3 fc1 groups each)

F32 = mybir.dt.float32
F32R = mybir.dt.float32r
BF16 = mybir.dt.bfloat16
ALU = mybir.AluOpType
ACTF = mybir.ActivationFunctionType


def _split_oversized_waits(nc, max_waits=1):
    """The walrus build in this container accepts only one sync-wait per
    instruction.  Hoist excess on_wait entries onto inserted same-engine
    NoOp instructions placed just before (queue-order preserves semantics;
    a NoOp-with-wait stalls the queue without flushing the engine pipe)."""
    for f in nc.m.functions:
        for b in f.blocks:
            new_list, changed, ctr = [], False, 0
            for i in b.instructions:
                si = i.sync_info
                w = list(si.on_wait) if si is not None else []
                if len(w) > max_waits:
                    extra, keep = w[:-max_waits], w[-max_waits:]
                    for ci in range(0, len(extra), max_waits):
                        ctr += 1
                        d = mybir.InstNoOp(
                            name=f"{i.name}-wsplit{ctr}",
                            engine=i.engine,
                        )
                        d.sync_info = mybir.SyncInfo(
                            on_update=[], on_wait=extra[ci : ci + max_waits]
                        )
                        new_list.append(d)
                    si.on_wait = keep
                    changed = True
                new_list.append(i)
            if changed:
                b.instructions = new_list


def build_program(qmax: float, walrus_fixups: bool = True):
    """Build the per-core Bass program (same NEFF on all 8 cores)."""
    nc = bass.Bass("TRN2", target_bir_lowering=False, debug=False)

    # xt is typed float32r end-to-end (same 4-byte layout as f32): the walrus
    # verifier requires every producer reaching an FP32r matmul operand to
    # emit f32r, and an f32r-to-f32r DMA satisfies it with no conversion.
    xt_d = nc.dram_tensor("xt", (D, M_SHARD), F32R, kind="ExternalInput").ap()
    # w1t/w1r are f32r-typed for the same reason (raw f32 bits, no
    # conversion on the DMA; the PE truncates on read).
    w1t_d = nc.dram_tensor("w1t", (D, H), F32R, kind="ExternalInput").ap()
    w2t_d = nc.dram_tensor("w2t", (H, D), F32, kind="ExternalInput").ap()
    # b1 comes host-side pre-packed as [128, KH]: column t holds
    # b1[t*128:(t+1)*128]; b2 likewise as [128, KD].
    b1_d = nc.dram_tensor("b1", (128, KH), F32, kind="ExternalInput").ap()
    b2_d = nc.dram_tensor("b2", (128, KD), F32, kind="ExternalInput").ap()
    id_d = nc.dram_tensor("ident", (128, 128), F32, kind="ExternalInput").ap()
    # fc2 computes out.T (d on partitions); the host untransposes.
    out_d = nc.dram_tensor("outT", (D, M_SHARD), F32, kind="ExternalOutput").ap()

    with tile.TileContext(nc) as tc, ExitStack() as ctx:
        const = ctx.enter_context(tc.tile_pool(name="const", bufs=1))
        w1p = ctx.enter_context(tc.tile_pool(name="w1p", bufs=1))
        w2qp = ctx.enter_context(tc.tile_pool(name="w2qp", bufs=1))
        wstage = ctx.enter_context(tc.tile_pool(name="wstage", bufs=4))
        xstage = ctx.enter_context(tc.tile_pool(name="xstage", bufs=2))
        hpool = ctx.enter_context(tc.tile_pool(name="hpool", bufs=3))
        opool = ctx.enter_context(tc.tile_pool(name="opool", bufs=2))
        scal = ctx.enter_context(tc.tile_pool(name="scal", bufs=1))
        ps1 = ctx.enter_context(tc.tile_pool(name="ps1", bufs=3, space="PSUM"))
        ps2 = ctx.enter_context(tc.tile_pool(name="ps2", bufs=3, space="PSUM"))
        dram = ctx.enter_context(tc.tile_pool(name="dram", bufs=1, space="DRAM"))

        # ---------- setup ----------
        # ident leads (needed at ~28us for the PE warmup transposes); b1/b2
        # DMAs are deferred until after the w1 scan stream so they don't
        # delay its critical 26us.
        ident = const.tile([128, 128], F32, tag="ident")
        nc.sync.dma_start(ident[:], id_d[:])
        b1_pack = const.tile([128, KH], F32, tag="b1pack")
        b2_pack = const.tile([128, KD], F32, tag="b2pack")
        ones_row = const.tile([1, 128], F32, tag="ones_row")
        nc.vector.memset(ones_row[:], 1.0)
        c_pos = const.tile([128, 1], F32, tag="c_pos")
        nc.vector.memset(c_pos[:], C_RNE)
        c_neg = const.tile([128, 1], F32, tag="c_neg")
        nc.vector.memset(c_neg[:], -C_RNE)

        def cross_part_max(macc, tag, use_pe=True):
            """macc[128,1] -> global scalar max replicated to [128,1]; then
            scale = gmax/qmax, inv = 1/scale.  use_pe: exact PE f32
            transposes (fast, for w1 where the PE is idle anyway).  Else:
            Pool C-reduce + DMA partition-broadcast — slower but fully
            decoupled from the PE queue (for w2, whose finalize must not
            wait for a PE slot)."""
            gmax = scal.tile([128, 1], F32, tag=f"{tag}gmax")
            if use_pe:
                rps = ps2.tile([1, 128], F32, tag="redT", name=f"{tag}rps", bufs=1)
                nc.tensor.transpose(rps[:], macc[:], ident[:])
                mrow = scal.tile([1, 128], F32, tag=f"{tag}mrow")
                nc.vector.tensor_copy(mrow[:], rps[:])
                g11 = scal.tile([1, 1], F32, tag=f"{tag}g11")
                nc.vector.tensor_reduce(g11[:], mrow[:], axis=mybir.AxisListType.X, op=ALU.max)
                grow = scal.tile([1, 128], F32, tag=f"{tag}grow")
                nc.vector.tensor_scalar(grow[:], ones_row[:], g11[:], None, op0=ALU.mult)
                gps = ps2.tile([128, 1], F32, tag="redT", name=f"{tag}gps", bufs=1)
                nc.tensor.transpose(gps[:], grow[:], ident[:1, :1])
                nc.vector.tensor_copy(gmax[:], gps[:])
            else:
                g11 = scal.tile([1, 1], F32, tag=f"{tag}g11")
                nc.gpsimd.tensor_reduce(g11[:], macc[:],
                                        axis=mybir.AxisListType.C, op=ALU.max)
                grow = scal.tile([1, 128], F32, tag=f"{tag}grow")
                nc.vector.tensor_scalar(grow[:], ones_row[:], g11[:], None,
                                        op0=ALU.mult)
                # partition-broadcast via a DRAM bounce (stride-0 SBUF
                # partition APs are rejected; a DRAM row read back with the
                # dims swapped is a plain 128-descriptor gather)
                drow = dram.tile([1, 128], F32, tag=f"{tag}drow")
                nc.sync.dma_start(drow[:], grow[:])
                nc.sync.dma_start(gmax[:], drow[:].rearrange("a b -> b a"))
            # walrus rejects ALU divide in tensor_scalar; mult by 1/qmax
            # differs from max/qmax by <=1 ulp (negligible global scale shift).
            scale = scal.tile([128, 1], F32, tag=f"{tag}scale")
            nc.vector.tensor_scalar(scale[:], gmax[:], 1.0 / float(qmax), None, op0=ALU.mult)
            inv_s = scal.tile([128, 1], F32, tag=f"{tag}inv")
            rec = nc.vector.reciprocal(inv_s[:], scale[:])
            return scale, inv_s, rec

        # ---------- w1: DMA once (resident), abs-max scan behind the DMA ----
        # Per-slice reduces land in columns of one accumulator tile; a single
        # final X-reduce replaces a 24-op serial max chain on the critical path.
        w1r = [w1p.tile([128, H], F32R, tag=f"w1r{d}", name=f"w1r{d}")
               for d in range(KD)]
        n_sc = H // W1_SCAN_CHUNK
        m1all = scal.tile([128, KD * n_sc], F32, tag="q1macc_all")
        macc1 = scal.tile([128, 1], F32, tag="q1macc")
        w1_last_dma = None
        for d in range(KD):
            for j in range(n_sc):
                c0 = j * W1_SCAN_CHUNK
                w1_last_dma = nc.sync.dma_start(
                    w1r[d][:, c0 : c0 + W1_SCAN_CHUNK],
                    w1t_d[d * 128 : (d + 1) * 128, c0 : c0 + W1_SCAN_CHUNK],
                )
                i = d * n_sc + j
                nc.vector.tensor_reduce(
                    m1all[:, i : i + 1],
                    w1r[d][:, c0 : c0 + W1_SCAN_CHUNK].bitcast(F32),
                    axis=mybir.AxisListType.X, op=ALU.max,
                    apply_absolute_value=True,
                )
        nc.vector.tensor_reduce(macc1[:], m1all[:], axis=mybir.AxisListType.X,
                                op=ALU.max)
        # b1/b2 now (after the scan stream, well before first use)
        nc.sync.dma_start(b1_pack[:], b1_d[:])
        nc.sync.dma_start(b2_pack[:], b2_d[:])

        s1, inv_s1, inv1_ins = cross_part_max(macc1, "q1")

        # Dummy ident transposes gated on inv_s1: they run back-to-back
        # through the ~3us window where DVE computes the first quantize
        # round, so the PE's p-state ramp is fully warm (and unbroken) when
        # fc1's first matmul issues right behind them (results unused).
        for i in range(6):
            wps = ps2.tile([128, 128], F32, tag="warm", name=f"warm{i}", bufs=1)
            wtr = nc.tensor.transpose(wps[:], ident[:], ident[:])
            add_dep_helper(wtr.ins, inv1_ins.ins,
                           reason="warm PE ramp into fc1 start")

        # b1' = b1 / s1   (per-partition column layout [128, KH])
        b1s = const.tile([128, KH], F32, tag="b1s")
        nc.vector.tensor_scalar(b1s[:], b1_pack[:], inv_s1[:], None, op0=ALU.mult)

        # ---------- x block loads (SWDGE on the Pool ring) ----------
        def load_x_block(blk, eng=None):
            """x(b0)/x(b1) ride the SP/HWDGE queue: in-order emission after
            the w1 scan makes the ordering gate implicit and keeps the ~0.5us
            per-descriptor SWDGE cost off the Pool engine (which paces the
            quantize rounds).  x(b2)/x(b3) use the Pool ring instead: their
            DMAs block on the xstage slot WAR until fc1(b0)/fc1(b1) finish,
            and on the in-order SP queue that would head-of-line block the
            w2 requant stream behind them."""
            m0 = blk * 392
            eng = eng or nc.sync
            xs = []
            xdma = None
            for d in range(KD):
                xs_ = xstage.tile([128, 392], F32R, tag=f"xs{d}", name=f"xs{d}")
                xdma = eng.dma_start(
                    xs_[:], xt_d[d * 128 : (d + 1) * 128, m0 : m0 + 392])
                xs.append(xs_)
            return xs, xdma

        # x(b0) and x(b1) go out right behind the w1 scan stream (SP queue
        # order); later blocks' DMAs are emitted early too — the xstage slot
        # WAR (bufs=2) self-throttles them until fc1 releases the slot.
        x_tiles = [None] * len(M_BLOCKS)
        x_tiles[0], _ = load_x_block(0)
        x_tiles[1], x1_last_dma = load_x_block(1)

        # ---------- fc1 ----------
        def fc1_group(blk, t, xs):
            """One fc1 psum group: hT[t] = relu_bf16(contract_d(w1q, xT) + b1')."""
            ps = ps1.tile([128, 392], F32, tag="ps1", name="ps1")
            for d in range(KD):
                nc.tensor.matmul(
                    ps[:],
                    w1r[d][:, t * 128 : (t + 1) * 128],
                    xs[d][:],
                    start=(d == 0), stop=(d == KD - 1),
                )
            hh_ = hpool.tile([128, 392], BF16, tag=f"hh{t}", name=f"hh{t}")
            nc.scalar.activation(hh_[:], ps[:], ACTF.Relu, bias=b1s[:, t : t + 1])
            return hh_

        # ---- w1 in-place quantize (j-major, split DVE/Pool so the rate
        # roughly matches fc1 block-0's PE consumption), interleaved with
        # fc1(b0) groups.  ACT is left free for the fc1 epilogues. ----
        h_blocks = [None] * len(M_BLOCKS)
        h_blocks[0] = []
        h_blocks[1] = []
        n_qj = H // QJ
        for j in range(n_qj):
            c0 = j * QJ
            for d in range(KD):
                sl = w1r[d][:, c0 : c0 + QJ]
                # The w*inv+C intermediate needs full f32 mantissa, so it
                # goes through an f32 scratch; only the final subtract (an
                # exact small integer, immune to f32r truncation) writes the
                # f32r-typed resident tile — every writer of w1r is f32r,
                # which is what walrus' rounded-producer check wants.
                # Engine split DVE{0,1,2}/Pool{3,4}/ACT{5} paces each round at
                # ~2.7/2.3/2.7us against fc1(b0)'s 2.94us PE consumption.
                qtag = "qsV" if d < 3 else ("qsP" if d < 5 else "qsA")
                qs = scal.tile([128, QJ], F32, tag=qtag, name="qscratch",
                               bufs=1)
                if d == 5:
                    nc.scalar.activation(qs[:], sl.bitcast(F32), ACTF.Identity,
                                         bias=c_pos[:], scale=inv_s1[:])
                    nc.scalar.activation(sl, qs[:], ACTF.Identity,
                                         bias=c_neg[:])
                else:
                    eng = nc.vector if d < 3 else nc.gpsimd
                    eng.tensor_scalar(qs[:], sl.bitcast(F32), inv_s1[:], C_RNE,
                                      op0=ALU.mult, op1=ALU.add)
                    op2 = eng.tensor_scalar(sl, qs[:], C_RNE, None,
                                            op0=ALU.subtract)
                    if d < 3:
                        dve_q_last = op2
            # Both b0 and b1 groups consume this round: ~5.9us of PE work per
            # ~2.7us quantize round, so the PE (not the quantize) is the
            # pacer and round-boundary stalls vanish.
            for t in range(j * 3, j * 3 + 3):
                h_blocks[0].append(fc1_group(0, t, x_tiles[0]))
                h_blocks[1].append(fc1_group(1, t, x_tiles[1]))

        # ---------- w2 scan (DVE reduces; DMAs land early, reduces run once
        # DVE clears its w1-quantize share) ----------
        w2q = [w2qp.tile([128, D], BF16, tag=f"w2q{t}", name=f"w2q{t}")
               for t in range(KH)]
        m2all = scal.tile([128, KH], F32, tag="q2macc_all")
        macc2 = scal.tile([128, 1], F32, tag="q2macc")
        for t in range(KH):
            wst = wstage.tile([128, D], F32, tag="w2st", name="w2st")
            dma = nc.sync.dma_start(wst[:], w2t_d[t * 128 : (t + 1) * 128, :])
            if t < 4:
                # the first (slot-free) scan DMAs otherwise race x(b0)/x(b1)
                # for DMA bandwidth right when fc1 needs x
                add_dep_helper(dma.ins, x1_last_dma.ins,
                               reason="w2 scan after x0/x1 streams")
            red = nc.vector.tensor_reduce(m2all[:, t : t + 1], wst[:],
                                          axis=mybir.AxisListType.X,
                                          op=ALU.max, apply_absolute_value=True)
            # keep the scheduler from interleaving these 0.86us reduces into
            # the DVE quantize rounds that pace fc1(b0)/fc1(b1)
            add_dep_helper(red.ins, dve_q_last.ins,
                           reason="w2 scan reduces after w1 quantize (DVE)")
        nc.vector.tensor_reduce(macc2[:], m2all[:], axis=mybir.AxisListType.X,
                                op=ALU.max)

        # w2's max finalize avoids the PE (no free PE slot until the rounds
        # end ~80us; macc2 is ready ~67us and requant wants inv_s2 ASAP).
        s2, inv_s2, _ = cross_part_max(macc2, "q2", use_pe=False)
        # c = s1 * s2  (final output scale), per-partition [128,1]
        cscale = scal.tile([128, 1], F32, tag="cscale")
        nc.vector.tensor_tensor(cscale[:], s1[:], s2[:], op=ALU.mult)

        # x(b2) DMA pushes go out on the Pool ring (emitted after the g11
        # reduce above); the xstage slot WAR holds them until fc1(b0)
        # finishes with x(b0).
        x_tiles[2], _ = load_x_block(2, eng=nc.gpsimd)

        # w2 pass 2: re-DMA (prefetches through the wstage ring as scan slots
        # free up) and quantize to bf16 (ints exact).  All ops on DVE: it is
        # free from ~70us and its tensor_scalar runs 2.5x faster than Pool's.
        for t in range(KH):
            wst2 = wstage.tile([128, D], F32, tag="w2st", name="w2st2")
            nc.sync.dma_start(wst2[:], w2t_d[t * 128 : (t + 1) * 128, :])
            nc.vector.tensor_scalar(wst2[:], wst2[:], inv_s2[:], C_RNE,
                                    op0=ALU.mult, op1=ALU.add)
            nc.vector.tensor_scalar(w2q[t][:], wst2[:], C_RNE, None,
                                    op0=ALU.subtract)

        # ---------- fc2 ----------
        def fc2_block(blk, split_last=False):
            """fc2 (transposed): outT[d, m] = c * contract_h(w2q, hT) + b2.
            split_last halves the final psum group along m so its epilogue
            and out-DMA overlap the PE instead of serializing after it."""
            m0 = blk * 392
            hh = h_blocks[blk]
            for dt in range(KD):
                halves = ([(0, 196), (196, 196)]
                          if (split_last and dt == KD - 1) else [(0, 392)])
                for mo, mw in halves:
                    ps_ = ps2.tile([128, 392], F32, tag="ps2", name="ps2")
                    for t in range(KH):
                        nc.tensor.matmul(
                            ps_[:, :mw],
                            w2q[t][:, dt * 128 : (dt + 1) * 128],
                            hh[t][:, mo : mo + mw],
                            start=(t == 0), stop=(t == KH - 1),
                        )
                    ot = opool.tile([128, 392], F32, tag="ot", name="ot")
                    # out = Identity(psum * c + b2)  — one ACT op
                    nc.scalar.activation(
                        ot[:, :mw], ps_[:, :mw], ACTF.Identity,
                        bias=b2_pack[:, dt : dt + 1], scale=cscale[:],
                    )
                    nc.sync.dma_start(
                        out_d[dt * 128 : (dt + 1) * 128, m0 + mo : m0 + mo + mw],
                        ot[:, :mw],
                    )

        # ---------- remaining schedule ----------
        h_blocks[2] = [fc1_group(2, t, x_tiles[2]) for t in range(KH)]
        x_tiles[3], _ = load_x_block(3, eng=nc.gpsimd)
        fc2_block(0)
        h_blocks[3] = [fc1_group(3, t, x_tiles[3]) for t in range(KH)]
        fc2_block(1)
        fc2_block(2)
        fc2_block(3, split_last=True)

    if walrus_fixups:
        _split_oversized_waits(nc)
    return nc


_PROGRAM_CACHE = {}


def _get_program(qmax: float):
    key = qmax
    if key not in _PROGRAM_CACHE:
        _PROGRAM_CACHE[key] = build_program(qmax)
    return _PROGRAM_CACHE[key]


def kernel(x, w1, b1, w2, b2, bits):
    qmax = float(2.0 ** (int(bits) - 1) - 1.0)
    nc = _get_program(qmax)

    x = np.ascontiguousarray(np.asarray(x, dtype=np.float32)).reshape(M_TOTAL, D)
    w1t = np.ascontiguousarray(np.asarray(w1, dtype=np.float32).T)   # [768, 3072]
    w2t = np.ascontiguousarray(np.asarray(w2, dtype=np.float32).T)   # [3072, 768]
    b1h = np.ascontiguousarray(
        np.asarray(b1, dtype=np.float32).reshape(KH, 128).T
    )  # [128, KH]
    b2h = np.ascontiguousarray(
        np.asarray(b2, dtype=np.float32).reshape(KD, 128).T
    )  # [128, KD]
    xt_full = np.ascontiguousarray(x.T)                              # [768, 12544]

    ident = np.eye(128, dtype=np.float32)
    in_maps = []
    for c in range(N_CORES):
        xt_c = np.ascontiguousarray(xt_full[:, c * M_SHARD : (c + 1) * M_SHARD])
        in_maps.append(
            {"xt": xt_c, "w1t": w1t, "w2t": w2t, "b1": b1h, "b2": b2h,
             "ident": ident}
        )

    res = bass_utils.run_bass_kernel_spmd(nc, in_maps, core_ids=list(range(N_CORES)))
    out = np.concatenate(
        [res.results[c]["outT"].T for c in range(N_CORES)], axis=0
    )
    return np.ascontiguousarray(out.reshape(B, S, D))
